# revision 2
# baseline (speedup 1.0000x reference)
"""Trainium2 Bass kernel for MiLoLinear: out = x @ (dequant4(W_q) + U@V).T + bias.

Strategy: dequant (4-bit unpack + affine) and the low-rank U@V correction are
folded on the HOST into a single bf16 W_eff [11008, 4096]; bias is added on
the host after gather. The device runs a pure column-parallel GEMM: core k
computes out[:, k*1376:(k+1)*1376] = x @ W_eff_k.T.

Per-core device program (column-split halves):
  half 0 accumulates cols [0,688) for all four 128-row s-blocks, half 1 cols
  [688,1376). PSUM = 4 tiles x [128,688] f32 (2 banks each) = all 8 banks.
  Half 0 only needs w cols [0,688) (5.65MB) + xt (4.2MB) -> the DMA stream
  (~260GB/s sustained) stays ahead of the PE by construction; w cols
  [688,1376) arrive during half 0 with the leftover bandwidth.
  Stationary = x tile [128c,128s] (one LDWEIGHTS per (half,t,st)); moving =
  w[:, t*1376 + half*688 +(0:512|512:688)]. Chunked PSUM drains (Act copy to
  bf16 + gpsimd DMA per chunk) overlap the tail.
"""

import sys

for _p in ("/opt/trn_rl_repo", "/root/.axon_site/_ro/trn_rl_repo"):
    if _p not in sys.path:
        sys.path.append(_p)

import numpy as np
import ml_dtypes

import concourse.bass as bass
import concourse.tile as tile
from concourse import bacc, mybir
from concourse.bass_utils import run_bass_kernel_spmd

OUT_F, IN_F, GROUP = 11008, 4096, 64
S = 512                              # rows of x
NCORES = 8
NKT = IN_F // 128                    # 32 contraction tiles
OL = OUT_F // NCORES                 # 1376 local output columns
NST = S // 128                       # 4 s-blocks
HC = OL // 2                         # 688 cols per half
HCHUNKS = [(0, 512), (512, HC)]

BF16 = ml_dtypes.bfloat16


def _build_program():
    nc = bacc.Bacc("TRN2", target_bir_lowering=False, debug=False)
    dt = mybir.dt

    # w host layout: [half, t-pair, 128, t_in_pair*688]: half-major so the
    # DMA stream delivers all of half 0's columns (5.65MB) first, then
    # half 1's -- the half-0 PE demand (~260GB/s with xt) is met by
    # construction and half 1's w arrives with the leftover bandwidth.
    w_in = nc.declare_dram_parameter("w", [2, NKT // 2, 128, 2 * HC], dt.bfloat16, isOutput=False)
    xt_in = nc.declare_dram_parameter("xt", [128, NKT * S], dt.bfloat16, isOutput=False)
    out_d = nc.declare_dram_parameter("out", [NST, 128, OL], dt.bfloat16, isOutput=True)

    with tile.TileContext(nc) as tc:
        with (
            tc.tile_pool(name="const", bufs=1) as cpool,
            tc.tile_pool(name="out", bufs=8) as outp,
            tc.tile_pool(name="ps", bufs=4, space="PSUM") as psp,
        ):
            # ---- input DMAs ----
            xt = cpool.tile([128, NKT * S], dt.bfloat16)
            for i in range(8):
                t = i * 4
                nc.sync.dma_start(xt[:, t * S:(t + 4) * S],
                                  xt_in[:, t * S:(t + 4) * S])
            # w SBUF free-dim layout per partition: [t, half, 688]
            w = cpool.tile([128, NKT, 2, HC], dt.bfloat16)
            for half in range(2):
                for tp in range(NKT // 2):
                    q = nc.scalar if tp % 2 == 0 else nc.gpsimd
                    q.dma_start(w[:, 2 * tp:2 * tp + 2, half, :],
                                w_in[half, tp])

            # ---- PE warmup (p-state ramp) while DMAs stream; writes are
            # start=True overwrites into ps0's bank, discarded by the real
            # start=True at t=0 ----
            wu = cpool.tile([128, 512], dt.bfloat16)
            nc.vector.memset(wu[:], 1.0)

            pss = [psp.tile([128, HC], dt.float32, tag="ps",
                            name=f"ps{i}") for i in range(NST)]
            for i in range(6):
                nc.tensor.matmul(pss[0][:, 0:512], wu[:, 0:128], wu[:],
                                 start=True, stop=True, skip_group_check=True)

            # ---- main GEMM: halves over columns ----
            for half in range(2):
                for t in range(NKT):
                    for st in range(NST):
                        lhs = xt[:, t * S + st * 128: t * S + (st + 1) * 128]
                        for a, b in HCHUNKS:
                            nc.tensor.matmul(
                                pss[st][:, a:b], lhs,
                                w[:, t, half, a:b],
                                start=(t == 0), stop=(t == NKT - 1))
                # chunked drain: per chunk, PSUM->SBUF copy split between the
                # Act and DVE engines, then DMA split over two issue queues,
                # so the tail is not serialized on any single engine
                for st in range(NST):
                    ot = outp.tile([128, HC], dt.bfloat16, tag="out")
                    for ci, (a, b) in enumerate(HCHUNKS):
                        if (st + ci) % 2 == 0:
                            nc.scalar.copy(ot[:, a:b], pss[st][:, a:b])
                            nc.gpsimd.dma_start(
                                out_d[st][:, half * HC + a:half * HC + b],
                                ot[:, a:b])
                        else:
                            nc.vector.tensor_copy(ot[:, a:b], pss[st][:, a:b])
                            nc.sync.dma_start(
                                out_d[st][:, half * HC + a:half * HC + b],
                                ot[:, a:b])
                if half == 0:
                    pss = [psp.tile([128, HC], dt.float32, tag="ps",
                                    name=f"ps1{i}") for i in range(NST)]

    nc.compile()
    return nc


def _prep_w(W_q, scale, zero, U, V):
    """Host: dequant + low-rank fold -> per-core [2, NKT//2, 128, 2*HC]."""
    Wq = W_q.astype(np.uint8)
    hi = (Wq >> 4).astype(np.float32)
    lo = (Wq & 0xF).astype(np.float32)
    Wg = np.concatenate([hi, lo], axis=0)            # [64, G]
    W = (Wg - zero) * scale
    W = W.reshape(OUT_F, IN_F)
    W += U.astype(np.float32) @ V.astype(np.float32)
    Wt = W.T.astype(BF16)                            # [IN_F, OUT_F]
    # [in, out_local] -> [tp, t2, p, half, hc] -> [half, tp, p, t2, hc]
    return [np.ascontiguousarray(
        Wt[:, k * OL:(k + 1) * OL]
        .reshape(NKT // 2, 2, 128, 2, HC)
        .transpose(3, 0, 2, 1, 4).reshape(2, NKT // 2, 128, 2 * HC))
        for k in range(NCORES)]


_CACHE = {}


def kernel(x, W_q, scale, zero, U, V, bias):
    x = np.asarray(x)
    W_q = np.asarray(W_q)
    scale = np.asarray(scale)
    zero = np.asarray(zero)
    U = np.asarray(U)
    V = np.asarray(V)
    bias = np.asarray(bias)

    if "nc" not in _CACHE:
        _CACHE["nc"] = _build_program()
    nc = _CACHE["nc"]

    # xt[p, t*S+s] = x[s, t*128+p]
    xt = np.ascontiguousarray(
        x.T.reshape(NKT, 128, S).transpose(1, 0, 2).reshape(128, NKT * S)
    ).astype(BF16)
    w_slabs = _prep_w(W_q, scale, zero, U, V)
    in_maps = [{"w": w_slabs[k], "xt": xt} for k in range(NCORES)]

    res = run_bass_kernel_spmd(nc, in_maps, list(range(NCORES)))

    out = np.empty((S, OUT_F), dtype=np.float32)
    for k in range(NCORES):
        oc = res.results[k]["out"].reshape(S, OL).astype(np.float32)
        out[:, k * OL:(k + 1) * OL] = oc
    out += bias.astype(np.float32)[None, :]
    return out
